# revision 6
# baseline (speedup 1.0000x reference)
"""NT-Xent loss kernel for Trainium2, 8-core SPMD.

Math (matches the reference exactly):
  reps = concat(z_i, z_j)                       [2B, C], B=4096, C=128
  rhat = reps / ||reps||                        (row L2 normalize)
  sim  = rhat @ rhat.T                          [2B, 2B]  (never materialized)
  pos_r = sim[r, (r+B) mod 2B]
  row logits = [pos_r, sim_r(with diag=-inf)] / T,  T=0.5
  loss = mean_r( logsumexp(row) - pos_r/T )
       = mean_r( ln(S_r - exp(2*d_r) + exp(2*pos_r)) - 2*pos_r )
  where S_r = sum_c exp(2 * rhat_r . rhat_c)  (includes diag + pos once)
        d_r = rhat_r . rhat_r  (~1; cancels the masked diagonal term)

Sharding: each of 8 cores owns 1024 query rows (contiguous block k),
computes its partial sum of (lse_r - 2*pos_r); host sums 8 scalars / 2B.
Core k's positive-partner block is block (k+4) % 8.
"""

import os

import numpy as np

import concourse.bacc as bacc
import concourse.bass as bass
import concourse.mybir as mybir
from concourse.bass_utils import run_bass_kernel_spmd
from concourse.masks import make_identity
from concourse.tile import TileContext

F32 = mybir.dt.float32
AF = mybir.ActivationFunctionType
ALU = mybir.AluOpType
AX = mybir.AxisListType

B = 4096
C = 128
TWOB = 2 * B            # 8192 total rows
N_CORES = 8
M_LOCAL = TWOB // N_CORES   # 1024 query rows per core
MT = M_LOCAL // 128         # 8 m-tiles of 128 queries
KT = TWOB // 128            # 64 key tiles of 128 rows
SPAN = 2048                 # ScalarE exp span = 4 PSUM banks
NG = TWOB // SPAN           # 4 column groups
NCHUNK = 8                  # keys DMA chunks
TPC = KT // NCHUNK          # 8 key tiles per chunk
ISCALE = 2.0                # 1 / temperature


def _normalize_block(nc, pool, x3, xn3, nt, name):
    """Row-normalize nt [128,128] tiles: xn3 = x3 / ||row||.

    Writing to a separate buffer (not in place) keeps downstream PE
    transposes to <=2 sem waits (LDWEIGHTS has very few wait slots).
    Returns inv [128, nt] = 1/||row|| (rsqrt via exp(-0.5*ln(x)): stays in
    the natural_log_exp ACT table set; Rsqrt is banned in bass).
    """
    sq = pool.tile([128, nt, C], F32, tag=f"{name}_sq")
    nrm = pool.tile([128, nt], F32, tag=f"{name}_nrm")
    inv = pool.tile([128, nt], F32, tag=f"{name}_inv")
    nc.vector.tensor_mul(sq[:], x3[:], x3[:])
    nc.vector.reduce_sum(nrm[:], sq[:], axis=AX.X)
    nc.scalar.activation(nrm[:], nrm[:], AF.Ln)
    nc.scalar.activation(inv[:], nrm[:], AF.Exp, scale=-0.5)
    for t in range(nt):
        nc.vector.tensor_scalar_mul(xn3[:, t, :], x3[:, t, :], inv[:, t : t + 1])
    return inv


def build_bass() -> bass.Bass:
    nc = bacc.Bacc()
    keys = nc.dram_tensor("keys", [TWOB, C], F32, kind="ExternalInput")
    q = nc.dram_tensor("q", [M_LOCAL, C], F32, kind="ExternalInput")
    p = nc.dram_tensor("p", [M_LOCAL, C], F32, kind="ExternalInput")
    out = nc.dram_tensor("out", [1, 1], F32, kind="ExternalOutput")

    with TileContext(nc) as tc:
        with (
            tc.tile_pool(name="big", bufs=1) as big,
            tc.tile_pool(name="small", bufs=1) as small,
            tc.tile_pool(name="scr", bufs=2) as scr,
            tc.tile_pool(name="ps", bufs=2, space="PSUM") as psp,
        ):
            ident = small.tile([128, 128], F32)
            make_identity(nc, ident[:])
            ones = small.tile([128, 1], F32)
            nc.vector.memset(ones[:], 1.0)

            # ---- queue all input DMAs early (q, p first: needed first)
            qt3 = big.tile([128, MT, C], F32)
            pt3 = big.tile([128, MT, C], F32)
            kt3 = big.tile([128, KT, C], F32)
            nc.sync.dma_start(
                out=qt3[:], in_=q[:].rearrange("(t p) c -> p t c", p=128)
            )
            nc.sync.dma_start(
                out=pt3[:], in_=p[:].rearrange("(t p) c -> p t c", p=128)
            )
            for g in range(NCHUNK):
                nc.sync.dma_start(
                    out=kt3[:, g * TPC : (g + 1) * TPC, :],
                    in_=keys[g * (TPC * 128) : (g + 1) * (TPC * 128), :].rearrange(
                        "(t p) c -> p t c", p=128
                    ),
                )

            # ---- normalize q and p; compute pos & corr row-dots
            qn3 = big.tile([128, MT, C], F32)
            pn3 = big.tile([128, MT, C], F32)
            _normalize_block(nc, big, qt3, qn3, MT, "q")
            _normalize_block(nc, big, pt3, pn3, MT, "p")

            # pos_r = qhat . phat, corr_r = qhat . qhat (tensor_tensor_reduce
            # crashes the exec unit on this runtime -- use mul+reduce instead)
            pos = small.tile([128, MT], F32)
            corr = small.tile([128, MT], F32)
            prod = scr.tile([128, MT, C], F32, tag="prod")
            nc.vector.tensor_mul(prod[:], qn3[:], pn3[:])
            nc.vector.reduce_sum(pos[:], prod[:], axis=AX.X)
            prod2 = scr.tile([128, MT, C], F32, tag="prod")
            nc.vector.tensor_mul(prod2[:], qn3[:], qn3[:])
            nc.vector.reduce_sum(corr[:], prod2[:], axis=AX.X)

            # ---- transpose q tiles -> qT [128C, 1024]
            qT = big.tile([128, M_LOCAL], F32)
            for t in range(MT):
                pst = psp.tile([128, 128], F32, tag="ps")
                nc.tensor.transpose(pst[:], qn3[:, t, :], ident[:])
                nc.vector.tensor_copy(qT[:, t * 128 : (t + 1) * 128], pst[:])

            # ---- normalize keys per chunk, transpose -> keysT [128C, 8192]
            keysT = big.tile([128, TWOB], F32)
            kn3 = big.tile([128, KT, C], F32)
            ksq = big.tile([128, TPC, C], F32)
            knrm = small.tile([128, KT], F32)
            kinv = small.tile([128, KT], F32)
            for g in range(NCHUNK):
                s = slice(g * TPC, (g + 1) * TPC)
                nc.vector.tensor_mul(ksq[:], kt3[:, s, :], kt3[:, s, :])
                nc.vector.reduce_sum(knrm[:, s], ksq[:], axis=AX.X)
                nc.scalar.activation(knrm[:, s], knrm[:, s], AF.Ln)
                nc.scalar.activation(kinv[:, s], knrm[:, s], AF.Exp, scale=-0.5)
                for t in range(g * TPC, (g + 1) * TPC):
                    nc.vector.tensor_scalar_mul(
                        kn3[:, t, :], kt3[:, t, :], kinv[:, t : t + 1]
                    )
                    pst = psp.tile([128, 128], F32, tag="ps")
                    nc.tensor.transpose(pst[:], kn3[:, t, :], ident[:])
                    nc.vector.tensor_copy(
                        keysT[:, t * 128 : (t + 1) * 128], pst[:]
                    )

            # ---- main loop: sim block matmuls + fused exp/row-sum on ScalarE
            acc = small.tile([128, MT * NG], F32)
            for g in range(NG):
                for m in range(MT):
                    psm = psp.tile([128, SPAN], F32, tag="ps")
                    for j in range(SPAN // 512):
                        col = g * SPAN + j * 512
                        nc.tensor.matmul(
                            psm[:, j * 512 : (j + 1) * 512],
                            lhsT=qT[:, m * 128 : (m + 1) * 128],
                            rhs=keysT[:, col : col + 512],
                            start=True,
                            stop=True,
                        )
                    # exp(2*sim) over the whole 4-bank span; accum_out gives
                    # the per-query partial row sum for free
                    nc.scalar.activation(
                        psm[:],
                        psm[:],
                        AF.Exp,
                        scale=ISCALE,
                        accum_out=acc[:, m * NG + g : m * NG + g + 1],
                    )

            # ---- finalize: loss_r = ln(S - e^{2 corr} + e^{2 pos}) - 2 pos
            S = small.tile([128, MT], F32)
            nc.vector.reduce_sum(
                S[:], acc[:].rearrange("p (m g) -> p m g", g=NG), axis=AX.X
            )
            ecorr = small.tile([128, MT], F32)
            epos = small.tile([128, MT], F32)
            nc.scalar.activation(ecorr[:], corr[:], AF.Exp, scale=ISCALE)
            nc.scalar.activation(epos[:], pos[:], AF.Exp, scale=ISCALE)
            tot = small.tile([128, MT], F32)
            nc.vector.tensor_sub(tot[:], S[:], ecorr[:])
            nc.vector.tensor_add(tot[:], tot[:], epos[:])
            nc.scalar.activation(tot[:], tot[:], AF.Ln)
            rowloss = small.tile([128, MT], F32)
            # rowloss = (pos * -2) + ln(tot)
            nc.vector.scalar_tensor_tensor(
                out=rowloss[:],
                in0=pos[:],
                scalar=-ISCALE,
                in1=tot[:],
                op0=ALU.mult,
                op1=ALU.add,
            )
            rsum = small.tile([128, 1], F32)
            nc.vector.reduce_sum(rsum[:], rowloss[:], axis=AX.X)

            # partition-axis sum via ones-matmul -> [1,1]
            pf = psp.tile([1, 1], F32, tag="ps")
            nc.tensor.matmul(pf[:], lhsT=rsum[:], rhs=ones[:], start=True, stop=True)
            outs = small.tile([1, 1], F32)
            nc.vector.tensor_copy(outs[:], pf[:])
            nc.sync.dma_start(out=out[:], in_=outs[:])

    nc.finalize()
    return nc


_NC_CACHE: bass.Bass | None = None
LAST_RESULTS = None  # BassKernelResults of the last run (for profiling)


def _get_nc() -> bass.Bass:
    global _NC_CACHE
    if _NC_CACHE is None:
        _NC_CACHE = build_bass()
    return _NC_CACHE


def kernel(z_i: np.ndarray, z_j: np.ndarray) -> np.ndarray:
    global LAST_RESULTS
    z_i = np.ascontiguousarray(np.asarray(z_i, dtype=np.float32))
    z_j = np.ascontiguousarray(np.asarray(z_j, dtype=np.float32))
    assert z_i.shape == (B, C) and z_j.shape == (B, C)

    reps = np.concatenate([z_i, z_j], axis=0)  # [2B, C]
    in_maps = []
    for k in range(N_CORES):
        kq = reps[k * M_LOCAL : (k + 1) * M_LOCAL]
        kp_blk = (k + N_CORES // 2) % N_CORES
        kp = reps[kp_blk * M_LOCAL : (kp_blk + 1) * M_LOCAL]
        in_maps.append(
            {
                "keys": reps,
                "q": np.ascontiguousarray(kq),
                "p": np.ascontiguousarray(kp),
            }
        )

    nc = _get_nc()
    trace = bool(int(os.environ.get("KERNEL_TRACE", "0")))
    res = run_bass_kernel_spmd(
        nc, in_maps, core_ids=list(range(N_CORES)), trace=trace
    )
    LAST_RESULTS = res
    total = sum(float(r["out"][0, 0]) for r in res.results)
    return np.float32(total / TWOB)


# revision 7
# speedup vs baseline: 1.4637x; 1.4637x over previous
"""NT-Xent loss kernel for Trainium2, 8-core SPMD.

Math (matches the reference exactly):
  reps = concat(z_i, z_j)                       [2B, C], B=4096, C=128
  rhat = reps / ||reps||                        (row L2 normalize)
  sim  = rhat @ rhat.T                          [2B, 2B]  (never materialized)
  pos_r = sim[r, (r+B) mod 2B]
  row logits = [pos_r, sim_r(with diag=-inf)] / T,  T=0.5
  loss = mean_r( logsumexp(row) - pos_r/T )
       = mean_r( ln(S_r - exp(2*d_r) + exp(2*pos_r)) - 2*pos_r )
  where S_r = sum_c exp(2 * rhat_r . rhat_c)  (includes diag + pos once)
        d_r = rhat_r . rhat_r  (~1; cancels the masked diagonal term)

Sharding: each of 8 cores owns 1024 query rows (contiguous block k),
computes its partial sum of (lse_r - 2*pos_r); host sums 8 scalars / 2B.
Core k's positive-partner block is block (k+4) % 8.

Perf design (v2):
  - matmul operands in float32r (~13-bit mantissa): 1 cyc/col vs 4 for
    fp32 -> 4x faster main loop, 2x faster PE transposes. pos/corr stay
    full fp32 on DVE; exp row-sums use ScalarE accum_out (free reduce).
  - one ACT table set (natural_log_exp_and_others) for Exp+Ln+rsqrt
    (rsqrt = exp(-0.5*ln)): a single ACT_TABLE_LOAD instead of 20.
  - norm squares on GpSimd (otherwise idle), reduces/scales/copies DVE.
  - 16 transposes share one 4-bank PSUM tile -> one wide DVE copy each.
  - col-group loop: scales -> transposes -> 8x(4 matmuls + exp span 2048)
    so group G+1 setup overlaps group G's ScalarE work.
"""

import os

import numpy as np

import concourse.bacc as bacc
import concourse.bass as bass
import concourse.mybir as mybir
from concourse.bass_utils import run_bass_kernel_spmd
from concourse.masks import make_identity
from concourse.tile import TileContext

F32 = mybir.dt.float32
F32R = mybir.dt.float32r
AF = mybir.ActivationFunctionType
ALU = mybir.AluOpType
AX = mybir.AxisListType

B = 4096
C = 128
TWOB = 2 * B            # 8192 total rows
N_CORES = 8
M_LOCAL = TWOB // N_CORES   # 1024 query rows per core
MT = M_LOCAL // 128         # 8 m-tiles of 128 queries
KT = TWOB // 128            # 64 key tiles of 128 rows
SPAN = 2048                 # ScalarE exp span = 4 PSUM banks
NG = TWOB // SPAN           # 4 column groups (16 key tiles each)
TPG = SPAN // 128           # 16 key tiles per column group
NCHUNK = 8                  # keys DMA chunks
TPC = KT // NCHUNK          # 8 key tiles per chunk
ISCALE = 2.0                # 1 / temperature


def _patch_act_tables():
    """Leave Exp/Ln only in natural_log_exp_and_others so bacc's greedy
    set chooser emits ONE table load for the whole kernel (measured: the
    default choice alternated exp<->ln sets, 21 loads, ~27us)."""
    if getattr(bacc, "_ntx_act_patched", False):
        return
    orig = bacc.get_activation_tables

    def patched(arch):
        out = {}
        for name, fns in orig(arch).items():
            if name != "natural_log_exp_and_others":
                fns = fns - {AF.Exp, AF.Ln}
            out[name] = fns
        return out

    bacc.get_activation_tables = patched
    bacc._ntx_act_patched = True


def build_bass() -> bass.Bass:
    _patch_act_tables()
    nc = bacc.Bacc()
    keys = nc.dram_tensor("keys", [TWOB, C], F32, kind="ExternalInput")
    q = nc.dram_tensor("q", [M_LOCAL, C], F32, kind="ExternalInput")
    p = nc.dram_tensor("p", [M_LOCAL, C], F32, kind="ExternalInput")
    out = nc.dram_tensor("out", [1, 1], F32, kind="ExternalOutput")

    with TileContext(nc) as tc:
        with (
            tc.tile_pool(name="big", bufs=1) as big,
            tc.tile_pool(name="small", bufs=1) as small,
            tc.tile_pool(name="scr", bufs=2) as scr,
            tc.tile_pool(name="ps", bufs=2, space="PSUM") as psp,
        ):
            ident = small.tile([128, 128], F32)
            make_identity(nc, ident[:])
            identr = small.tile([128, 128], F32R)
            nc.vector.tensor_copy(identr[:], ident[:])  # round -> f32r
            ones = small.tile([128, 1], F32)
            nc.vector.memset(ones[:], 1.0)

            # ---- input DMAs (q, p first: needed first)
            qt3 = big.tile([128, MT, C], F32)
            pt3 = big.tile([128, MT, C], F32)
            kt3 = big.tile([128, KT, C], F32)
            nc.sync.dma_start(
                out=qt3[:], in_=q[:].rearrange("(t p) c -> p t c", p=128)
            )
            nc.sync.dma_start(
                out=pt3[:], in_=p[:].rearrange("(t p) c -> p t c", p=128)
            )
            for g in range(NCHUNK):
                nc.sync.dma_start(
                    out=kt3[:, g * TPC : (g + 1) * TPC, :],
                    in_=keys[g * (TPC * 128) : (g + 1) * (TPC * 128), :].rearrange(
                        "(t p) c -> p t c", p=128
                    ),
                )

            # ---- row norms: squares on GpSimd, reduce on DVE
            # nrm columns: 0:8 q, 8:16 p, 16:80 keys chunks
            nrm = small.tile([128, 16 + KT], F32)
            inv = small.tile([128, 16 + KT], F32)
            sqq = scr.tile([128, MT, C], F32, tag="sq")
            nc.gpsimd.tensor_mul(sqq[:], qt3[:], qt3[:])
            nc.vector.reduce_sum(nrm[:, 0:MT], sqq[:], axis=AX.X)
            sqp = scr.tile([128, MT, C], F32, tag="sq")
            nc.gpsimd.tensor_mul(sqp[:], pt3[:], pt3[:])
            nc.vector.reduce_sum(nrm[:, MT : 2 * MT], sqp[:], axis=AX.X)
            for g in range(NCHUNK):
                s = slice(g * TPC, (g + 1) * TPC)
                ksq = scr.tile([128, TPC, C], F32, tag="sq")
                nc.gpsimd.tensor_mul(ksq[:], kt3[:, s, :], kt3[:, s, :])
                nc.vector.reduce_sum(
                    nrm[:, 16 + g * TPC : 16 + (g + 1) * TPC], ksq[:], axis=AX.X
                )
                if g == 3:
                    # batch 1: q, p, keys chunks 0-3 (cols 0:48)
                    nc.scalar.activation(nrm[:, 0:48], nrm[:, 0:48], AF.Ln)
                    nc.scalar.activation(
                        inv[:, 0:48], nrm[:, 0:48], AF.Exp, scale=-0.5
                    )
            # batch 2: keys chunks 4-7 (cols 48:80)
            nc.scalar.activation(nrm[:, 48:80], nrm[:, 48:80], AF.Ln)
            nc.scalar.activation(inv[:, 48:80], nrm[:, 48:80], AF.Exp, scale=-0.5)

            # ---- normalize q (f32 for pos/corr) and p; round q -> f32r
            qn3 = big.tile([128, MT, C], F32)
            pn3 = big.tile([128, MT, C], F32)
            for t in range(MT):
                nc.vector.tensor_scalar_mul(
                    qn3[:, t, :], qt3[:, t, :], inv[:, t : t + 1]
                )
            for t in range(MT):
                nc.vector.tensor_scalar_mul(
                    pn3[:, t, :], pt3[:, t, :], inv[:, MT + t : MT + t + 1]
                )
            pos = small.tile([128, MT], F32)
            corr = small.tile([128, MT], F32)
            prod = scr.tile([128, MT, C], F32, tag="sq")
            nc.vector.tensor_mul(prod[:], qn3[:], pn3[:])
            nc.vector.reduce_sum(pos[:], prod[:], axis=AX.X)
            prod2 = scr.tile([128, MT, C], F32, tag="sq")
            nc.vector.tensor_mul(prod2[:], qn3[:], qn3[:])
            nc.vector.reduce_sum(corr[:], prod2[:], axis=AX.X)

            qn3r = big.tile([128, MT, C], F32R)
            nc.vector.tensor_copy(qn3r[:], qn3[:])

            # ---- qT via 8 transposes into one PSUM tile, one wide copy
            qT = big.tile([128, M_LOCAL], F32R)
            tq = psp.tile([128, SPAN], F32R, tag="ps")
            for t in range(MT):
                nc.tensor.transpose(
                    tq[:, t * 128 : (t + 1) * 128], qn3r[:, t, :], identr[:]
                )
            nc.vector.tensor_copy(qT[:], tq[:, 0:M_LOCAL])

            # ---- main: per column group: scale+transpose 16 key tiles,
            # then 8 m-tiles of (4 matmuls + fused exp/rowsum)
            keysT = big.tile([128, TWOB], F32R)
            kn3 = big.tile([128, KT, C], F32R)
            acc = small.tile([128, MT * NG], F32)
            for G in range(NG):
                for t in range(G * TPG, (G + 1) * TPG):
                    nc.vector.tensor_scalar_mul(
                        kn3[:, t, :], kt3[:, t, :], inv[:, 16 + t : 17 + t]
                    )
                tp = psp.tile([128, SPAN], F32R, tag="ps")
                for i, t in enumerate(range(G * TPG, (G + 1) * TPG)):
                    nc.tensor.transpose(
                        tp[:, i * 128 : (i + 1) * 128], kn3[:, t, :], identr[:]
                    )
                nc.vector.tensor_copy(
                    keysT[:, G * SPAN : (G + 1) * SPAN], tp[:]
                )
                for m in range(MT):
                    psm = psp.tile([128, SPAN], F32, tag="ps")
                    for j in range(SPAN // 512):
                        col = G * SPAN + j * 512
                        nc.tensor.matmul(
                            psm[:, j * 512 : (j + 1) * 512],
                            lhsT=qT[:, m * 128 : (m + 1) * 128],
                            rhs=keysT[:, col : col + 512],
                            start=True,
                            stop=True,
                        )
                    nc.scalar.activation(
                        psm[:],
                        psm[:],
                        AF.Exp,
                        scale=ISCALE,
                        accum_out=acc[:, m * NG + G : m * NG + G + 1],
                    )

            # ---- finalize: loss_r = ln(S - e^{2 corr} + e^{2 pos}) - 2 pos
            S = small.tile([128, MT], F32)
            nc.vector.reduce_sum(
                S[:], acc[:].rearrange("p (m g) -> p m g", g=NG), axis=AX.X
            )
            ecorr = small.tile([128, MT], F32)
            epos = small.tile([128, MT], F32)
            nc.scalar.activation(ecorr[:], corr[:], AF.Exp, scale=ISCALE)
            nc.scalar.activation(epos[:], pos[:], AF.Exp, scale=ISCALE)
            tot = small.tile([128, MT], F32)
            nc.vector.tensor_sub(tot[:], S[:], ecorr[:])
            nc.vector.tensor_add(tot[:], tot[:], epos[:])
            nc.scalar.activation(tot[:], tot[:], AF.Ln)
            rowloss = small.tile([128, MT], F32)
            nc.vector.scalar_tensor_tensor(
                out=rowloss[:],
                in0=pos[:],
                scalar=-ISCALE,
                in1=tot[:],
                op0=ALU.mult,
                op1=ALU.add,
            )
            rsum = small.tile([128, 1], F32)
            nc.vector.reduce_sum(rsum[:], rowloss[:], axis=AX.X)

            pf = psp.tile([1, 1], F32, tag="ps")
            nc.tensor.matmul(pf[:], lhsT=rsum[:], rhs=ones[:], start=True, stop=True)
            outs = small.tile([1, 1], F32)
            nc.vector.tensor_copy(outs[:], pf[:])
            nc.sync.dma_start(out=out[:], in_=outs[:])

    nc.finalize()
    return nc


_NC_CACHE: bass.Bass | None = None
LAST_RESULTS = None  # BassKernelResults of the last run (for profiling)


def _get_nc() -> bass.Bass:
    global _NC_CACHE
    if _NC_CACHE is None:
        _NC_CACHE = build_bass()
    return _NC_CACHE


def kernel(z_i: np.ndarray, z_j: np.ndarray) -> np.ndarray:
    global LAST_RESULTS
    z_i = np.ascontiguousarray(np.asarray(z_i, dtype=np.float32))
    z_j = np.ascontiguousarray(np.asarray(z_j, dtype=np.float32))
    assert z_i.shape == (B, C) and z_j.shape == (B, C)

    reps = np.concatenate([z_i, z_j], axis=0)  # [2B, C]
    in_maps = []
    for k in range(N_CORES):
        kq = reps[k * M_LOCAL : (k + 1) * M_LOCAL]
        kp_blk = (k + N_CORES // 2) % N_CORES
        kp = reps[kp_blk * M_LOCAL : (kp_blk + 1) * M_LOCAL]
        in_maps.append(
            {
                "keys": reps,
                "q": np.ascontiguousarray(kq),
                "p": np.ascontiguousarray(kp),
            }
        )

    nc = _get_nc()
    trace = bool(int(os.environ.get("KERNEL_TRACE", "0")))
    res = run_bass_kernel_spmd(
        nc, in_maps, core_ids=list(range(N_CORES)), trace=trace
    )
    LAST_RESULTS = res
    total = sum(float(r["out"][0, 0]) for r in res.results)
    return np.float32(total / TWOB)


# revision 8
# speedup vs baseline: 1.4777x; 1.0095x over previous
"""NT-Xent loss kernel for Trainium2, 8-core SPMD.

Math (matches the reference exactly):
  reps = concat(z_i, z_j)                       [2B, C], B=4096, C=128
  rhat = reps / ||reps||                        (row L2 normalize)
  sim  = rhat @ rhat.T                          [2B, 2B]  (never materialized)
  pos_r = sim[r, (r+B) mod 2B]
  row logits = [pos_r, sim_r(with diag=-inf)] / T,  T=0.5
  loss = mean_r( logsumexp(row) - pos_r/T )
       = mean_r( ln(S_r - exp(2*d_r) + exp(2*pos_r)) - 2*pos_r )
  where S_r = sum_c exp(2 * rhat_r . rhat_c)  (includes diag + pos once)
        d_r = rhat_r . rhat_r  (~1; cancels the masked diagonal term)

Sharding: each of 8 cores owns 1024 query rows (contiguous block k) and
computes per-row (lse_r - 2*pos_r), reduced on-device to a [128,1]
per-partition partial; host sums 8x128 values / 2B. Core k's positive
partner block is block (k+4) % 8.

Perf design (v3, ~measured 138us at v2 -> targeting <110us):
  - matmul operands in float32r (~13-bit mantissa): 1 cyc/col vs 4 for
    fp32 -> 4x faster main loop, 2x faster PE transposes. pos/corr stay
    full fp32 on DVE; exp row-sums use ScalarE accum_out (free reduce).
  - one ACT table set (natural_log_exp_and_others) for Exp+Ln+rsqrt
    (rsqrt = exp(-0.5*ln)): a single ACT_TABLE_LOAD instead of 20.
  - head critical path: DMA order q,k0..k7,p; q/k0/k1 squares on DVE,
    rest on GpSimd; rsqrt in 5 small batches so column group 0 starts
    ~15us in instead of 35us. p/pos/corr processing runs during the
    main loop (only needed at the end).
  - 16 transposes share one 4-bank PSUM tile -> one wide DVE copy each.
  - col-group loop: scales -> transposes -> 8x(4 matmuls + exp span 2048).
"""

import os

import numpy as np

import concourse.bacc as bacc
import concourse.bass as bass
import concourse.mybir as mybir
from concourse.bass_utils import run_bass_kernel_spmd
from concourse.masks import make_identity
from concourse.tile import TileContext

F32 = mybir.dt.float32
F32R = mybir.dt.float32r
AF = mybir.ActivationFunctionType
ALU = mybir.AluOpType
AX = mybir.AxisListType

B = 4096
C = 128
TWOB = 2 * B            # 8192 total rows
N_CORES = 8
M_LOCAL = TWOB // N_CORES   # 1024 query rows per core
MT = M_LOCAL // 128         # 8 m-tiles of 128 queries
KT = TWOB // 128            # 64 key tiles of 128 rows
SPAN = 2048                 # ScalarE exp span = 4 PSUM banks
NG = TWOB // SPAN           # 4 column groups (16 key tiles each)
TPG = SPAN // 128           # 16 key tiles per column group
NCHUNK = 8                  # keys DMA chunks
TPC = KT // NCHUNK          # 8 key tiles per chunk
ISCALE = 2.0                # 1 / temperature

# nrm/inv column layout: 0:8 q | 8:72 keys (chunk g at 8+g*8) | 72:80 p
QC = 0
KC = MT
PC = MT + KT


def _patch_act_tables():
    """Leave Exp/Ln only in natural_log_exp_and_others so bacc's greedy
    set chooser emits ONE table load for the whole kernel (measured: the
    default choice alternated exp<->ln sets, 21 loads, ~27us)."""
    if getattr(bacc, "_ntx_act_patched", False):
        return
    orig = bacc.get_activation_tables

    def patched(arch):
        out = {}
        for name, fns in orig(arch).items():
            if name != "natural_log_exp_and_others":
                fns = fns - {AF.Exp, AF.Ln}
            out[name] = fns
        return out

    bacc.get_activation_tables = patched
    bacc._ntx_act_patched = True


def build_bass() -> bass.Bass:
    _patch_act_tables()
    nc = bacc.Bacc()
    keys = nc.dram_tensor("keys", [TWOB, C], F32, kind="ExternalInput")
    q = nc.dram_tensor("q", [M_LOCAL, C], F32, kind="ExternalInput")
    p = nc.dram_tensor("p", [M_LOCAL, C], F32, kind="ExternalInput")
    out = nc.dram_tensor("out", [128, 1], F32, kind="ExternalOutput")

    with TileContext(nc) as tc:
        with (
            tc.tile_pool(name="big", bufs=1) as big,
            tc.tile_pool(name="small", bufs=1) as small,
            tc.tile_pool(name="scr", bufs=2) as scr,
            tc.tile_pool(name="ps", bufs=2, space="PSUM") as psp,
        ):
            ident = small.tile([128, 128], F32)
            make_identity(nc, ident[:])
            identr = small.tile([128, 128], F32R)
            nc.vector.tensor_copy(identr[:], ident[:])  # round -> f32r

            # ---- input DMAs: q then keys chunks then p (p not on the
            # critical path -- only needed for the final pos term)
            qt3 = big.tile([128, MT, C], F32)
            pt3 = big.tile([128, MT, C], F32)
            kt3 = big.tile([128, KT, C], F32)
            nc.sync.dma_start(
                out=qt3[:], in_=q[:].rearrange("(t p) c -> p t c", p=128)
            )
            for g in range(NCHUNK):
                nc.sync.dma_start(
                    out=kt3[:, g * TPC : (g + 1) * TPC, :],
                    in_=keys[g * (TPC * 128) : (g + 1) * (TPC * 128), :].rearrange(
                        "(t p) c -> p t c", p=128
                    ),
                )
            nc.sync.dma_start(
                out=pt3[:], in_=p[:].rearrange("(t p) c -> p t c", p=128)
            )

            nrm = small.tile([128, 16 + KT], F32)
            inv = small.tile([128, 16 + KT], F32)

            def norms(x3, col, n, engine):
                sq = scr.tile([128, n, C], F32, tag="sq")
                engine.tensor_mul(sq[:], x3[:], x3[:])
                nc.vector.reduce_sum(nrm[:, col : col + n], sq[:], axis=AX.X)

            def rsqrt_batch(col, n):
                nc.scalar.activation(nrm[:, col : col + n], nrm[:, col : col + n], AF.Ln)
                nc.scalar.activation(
                    inv[:, col : col + n], nrm[:, col : col + n], AF.Exp, scale=-0.5
                )

            # head-critical norms on DVE, the rest on otherwise-idle GpSimd
            norms(qt3, QC, MT, nc.vector)
            norms(kt3[:, 0:TPC, :], KC, TPC, nc.vector)
            norms(kt3[:, TPC : 2 * TPC, :], KC + TPC, TPC, nc.vector)
            rsqrt_batch(QC, 24)  # q + chunks 0,1 -> group 0 can start
            for g in range(2, NCHUNK):
                norms(kt3[:, g * TPC : (g + 1) * TPC, :], KC + g * TPC, TPC, nc.gpsimd)
                if g % 2 == 1:
                    rsqrt_batch(KC + (g - 1) * TPC, 2 * TPC)

            # ---- normalize q (f32 kept for pos/corr), round to f32r, qT
            qn3 = big.tile([128, MT, C], F32)
            for t in range(MT):
                nc.vector.tensor_scalar_mul(
                    qn3[:, t, :], qt3[:, t, :], inv[:, t : t + 1]
                )
            qn3r = big.tile([128, MT, C], F32R)
            nc.vector.tensor_copy(qn3r[:], qn3[:])
            qT = big.tile([128, M_LOCAL], F32R)
            tq = psp.tile([128, SPAN], F32R, tag="ps")
            for t in range(MT):
                nc.tensor.transpose(
                    tq[:, t * 128 : (t + 1) * 128], qn3r[:, t, :], identr[:]
                )
            nc.vector.tensor_copy(qT[:], tq[:, 0:M_LOCAL])

            # ---- main: per column group: scale+transpose 16 key tiles,
            # then 8 m-tiles of (4 matmuls + fused exp/rowsum)
            keysT = big.tile([128, TWOB], F32R)
            kn3 = big.tile([128, KT, C], F32R)
            acc = small.tile([128, MT * NG], F32)
            for G in range(NG):
                for t in range(G * TPG, (G + 1) * TPG):
                    nc.vector.tensor_scalar_mul(
                        kn3[:, t, :], kt3[:, t, :], inv[:, KC + t : KC + t + 1]
                    )
                tp = psp.tile([128, SPAN], F32R, tag="ps")
                for i, t in enumerate(range(G * TPG, (G + 1) * TPG)):
                    nc.tensor.transpose(
                        tp[:, i * 128 : (i + 1) * 128], kn3[:, t, :], identr[:]
                    )
                nc.vector.tensor_copy(
                    keysT[:, G * SPAN : (G + 1) * SPAN], tp[:]
                )
                for m in range(MT):
                    psm = psp.tile([128, SPAN], F32, tag="ps")
                    for j in range(SPAN // 512):
                        col = G * SPAN + j * 512
                        nc.tensor.matmul(
                            psm[:, j * 512 : (j + 1) * 512],
                            lhsT=qT[:, m * 128 : (m + 1) * 128],
                            rhs=keysT[:, col : col + 512],
                            start=True,
                            stop=True,
                        )
                    nc.scalar.activation(
                        psm[:],
                        psm[:],
                        AF.Exp,
                        scale=ISCALE,
                        accum_out=acc[:, m * NG + G : m * NG + G + 1],
                    )

            # ---- p path (overlaps the main loop; needed only at the end)
            norms(pt3, PC, MT, nc.gpsimd)
            rsqrt_batch(PC, MT)
            pn3 = big.tile([128, MT, C], F32)
            for t in range(MT):
                nc.vector.tensor_scalar_mul(
                    pn3[:, t, :], pt3[:, t, :], inv[:, PC + t : PC + t + 1]
                )
            pos = small.tile([128, MT], F32)
            corr = small.tile([128, MT], F32)
            prod = scr.tile([128, MT, C], F32, tag="sq")
            nc.vector.tensor_mul(prod[:], qn3[:], pn3[:])
            nc.vector.reduce_sum(pos[:], prod[:], axis=AX.X)
            prod2 = scr.tile([128, MT, C], F32, tag="sq")
            nc.vector.tensor_mul(prod2[:], qn3[:], qn3[:])
            nc.vector.reduce_sum(corr[:], prod2[:], axis=AX.X)

            # ---- finalize: loss_r = ln(S - e^{2 corr} + e^{2 pos}) - 2 pos
            S = small.tile([128, MT], F32)
            nc.vector.reduce_sum(
                S[:], acc[:].rearrange("p (m g) -> p m g", g=NG), axis=AX.X
            )
            ecorr = small.tile([128, MT], F32)
            epos = small.tile([128, MT], F32)
            nc.scalar.activation(ecorr[:], corr[:], AF.Exp, scale=ISCALE)
            nc.scalar.activation(epos[:], pos[:], AF.Exp, scale=ISCALE)
            tot = small.tile([128, MT], F32)
            nc.vector.tensor_sub(tot[:], S[:], ecorr[:])
            nc.vector.tensor_add(tot[:], tot[:], epos[:])
            nc.scalar.activation(tot[:], tot[:], AF.Ln)
            rowloss = small.tile([128, MT], F32)
            nc.vector.scalar_tensor_tensor(
                out=rowloss[:],
                in0=pos[:],
                scalar=-ISCALE,
                in1=tot[:],
                op0=ALU.mult,
                op1=ALU.add,
            )
            rsum = small.tile([128, 1], F32)
            nc.vector.reduce_sum(rsum[:], rowloss[:], axis=AX.X)
            nc.sync.dma_start(out=out[:], in_=rsum[:])

    nc.finalize()
    return nc


_NC_CACHE: bass.Bass | None = None
LAST_RESULTS = None  # BassKernelResults of the last run (for profiling)


def _get_nc() -> bass.Bass:
    global _NC_CACHE
    if _NC_CACHE is None:
        _NC_CACHE = build_bass()
    return _NC_CACHE


def kernel(z_i: np.ndarray, z_j: np.ndarray) -> np.ndarray:
    global LAST_RESULTS
    z_i = np.ascontiguousarray(np.asarray(z_i, dtype=np.float32))
    z_j = np.ascontiguousarray(np.asarray(z_j, dtype=np.float32))
    assert z_i.shape == (B, C) and z_j.shape == (B, C)

    reps = np.concatenate([z_i, z_j], axis=0)  # [2B, C]
    in_maps = []
    for k in range(N_CORES):
        kq = reps[k * M_LOCAL : (k + 1) * M_LOCAL]
        kp_blk = (k + N_CORES // 2) % N_CORES
        kp = reps[kp_blk * M_LOCAL : (kp_blk + 1) * M_LOCAL]
        in_maps.append(
            {
                "keys": reps,
                "q": np.ascontiguousarray(kq),
                "p": np.ascontiguousarray(kp),
            }
        )

    nc = _get_nc()
    trace = bool(int(os.environ.get("KERNEL_TRACE", "0")))
    res = run_bass_kernel_spmd(
        nc, in_maps, core_ids=list(range(N_CORES)), trace=trace
    )
    LAST_RESULTS = res
    total = sum(float(r["out"].sum()) for r in res.results)
    return np.float32(total / TWOB)


# revision 11
# speedup vs baseline: 1.6327x; 1.1049x over previous
"""NT-Xent loss kernel for Trainium2, 8-core SPMD.

Math (matches the reference exactly):
  reps = concat(z_i, z_j)                       [2B, C], B=4096, C=128
  rhat = reps / ||reps||                        (row L2 normalize)
  sim  = rhat @ rhat.T                          [2B, 2B]  (never materialized)
  pos_r = sim[r, (r+B) mod 2B]
  row logits = [pos_r, sim_r(with diag=-inf)] / T,  T=0.5
  loss = mean_r( logsumexp(row) - pos_r/T )
       = mean_r( ln(S_r - exp(2*d_r) + exp(2*pos_r)) - 2*pos_r )
  where S_r = sum_c exp(2 * rhat_r . rhat_c)  (includes diag + pos once)
        d_r = rhat_r . rhat_r  (~1; cancels the masked diagonal term)

Sharding: each of 8 cores owns 1024 query rows (contiguous block k) and
computes per-row (lse_r - 2*pos_r), reduced on-device to a [128,1]
per-partition partial; host sums 8x128 values / 2B. Core k's positive
partner block is block (k+4) % 8.

Perf design (v3, ~measured 138us at v2 -> targeting <110us):
  - matmul operands in float32r (~13-bit mantissa): 1 cyc/col vs 4 for
    fp32 -> 4x faster main loop, 2x faster PE transposes. pos/corr stay
    full fp32 on DVE; exp row-sums use ScalarE accum_out (free reduce).
  - one ACT table set (natural_log_exp_and_others) for Exp+Ln+rsqrt
    (rsqrt = exp(-0.5*ln)): a single ACT_TABLE_LOAD instead of 20.
  - head critical path: DMA order q,k0..k7,p; q/k0/k1 squares on DVE,
    rest on GpSimd; rsqrt in 5 small batches so column group 0 starts
    ~15us in instead of 35us. p/pos/corr processing runs during the
    main loop (only needed at the end).
  - 16 transposes share one 4-bank PSUM tile -> one wide DVE copy each.
  - col-group loop: scales -> transposes -> 8x(4 matmuls + exp span 2048).
"""

import os

import numpy as np

import concourse.bacc as bacc
import concourse.bass as bass
import concourse.mybir as mybir
from concourse.bass_utils import run_bass_kernel_spmd
from concourse.masks import make_identity
from concourse.tile import TileContext

F32 = mybir.dt.float32
F32R = mybir.dt.float32r
AF = mybir.ActivationFunctionType
ALU = mybir.AluOpType
AX = mybir.AxisListType

B = 4096
C = 128
TWOB = 2 * B            # 8192 total rows
N_CORES = 8
M_LOCAL = TWOB // N_CORES   # 1024 query rows per core
MT = M_LOCAL // 128         # 8 m-tiles of 128 queries
KT = TWOB // 128            # 64 key tiles of 128 rows
SPAN = 2048                 # ScalarE exp span = 4 PSUM banks
NG = TWOB // SPAN           # 4 column groups (16 key tiles each)
TPG = SPAN // 128           # 16 key tiles per column group
NCHUNK = 8                  # keys DMA chunks
TPC = KT // NCHUNK          # 8 key tiles per chunk
ISCALE = 2.0                # 1 / temperature

# nrm/inv column layout: 0:8 q | 8:72 keys (chunk g at 8+g*8) | 72:80 p
QC = 0
KC = MT
PC = MT + KT


def _patch_act_tables():
    """Leave Exp/Ln only in natural_log_exp_and_others so bacc's greedy
    set chooser emits ONE table load for the whole kernel (measured: the
    default choice alternated exp<->ln sets, 21 loads, ~27us)."""
    if getattr(bacc, "_ntx_act_patched", False):
        return
    orig = bacc.get_activation_tables

    def patched(arch):
        out = {}
        for name, fns in orig(arch).items():
            if name != "natural_log_exp_and_others":
                fns = fns - {AF.Exp, AF.Ln}
            out[name] = fns
        return out

    bacc.get_activation_tables = patched
    bacc._ntx_act_patched = True


def build_bass() -> bass.Bass:
    _patch_act_tables()
    nc = bacc.Bacc()
    keys = nc.dram_tensor("keys", [TWOB, C], F32, kind="ExternalInput")
    q = nc.dram_tensor("q", [M_LOCAL, C], F32, kind="ExternalInput")
    p = nc.dram_tensor("p", [M_LOCAL, C], F32, kind="ExternalInput")
    out = nc.dram_tensor("out", [128, 1], F32, kind="ExternalOutput")

    with TileContext(nc) as tc:
        with (
            tc.tile_pool(name="big", bufs=1) as big,
            tc.tile_pool(name="small", bufs=1) as small,
            tc.tile_pool(name="scr", bufs=2) as scr,
            tc.tile_pool(name="scr1", bufs=1) as scr1,
            tc.tile_pool(name="ps", bufs=2, space="PSUM") as psp,
        ):
            ident = small.tile([128, 128], F32)
            make_identity(nc, ident[:])
            identr = small.tile([128, 128], F32R)
            nc.vector.tensor_copy(identr[:], ident[:])  # round -> f32r

            # ---- input DMAs: q then keys chunks then p (p not on the
            # critical path -- only needed for the final pos term)
            qt3 = big.tile([128, MT, C], F32)
            pt3 = big.tile([128, MT, C], F32)
            kt3 = big.tile([128, KT, C], F32)
            nc.sync.dma_start(
                out=qt3[:], in_=q[:].rearrange("(t p) c -> p t c", p=128)
            )
            for g in range(NCHUNK):
                nc.sync.dma_start(
                    out=kt3[:, g * TPC : (g + 1) * TPC, :],
                    in_=keys[g * (TPC * 128) : (g + 1) * (TPC * 128), :].rearrange(
                        "(t p) c -> p t c", p=128
                    ),
                )
            nc.sync.dma_start(
                out=pt3[:], in_=p[:].rearrange("(t p) c -> p t c", p=128)
            )

            nrm = small.tile([128, 16 + KT], F32)
            inv = small.tile([128, 16 + KT], F32)

            def norms(x3, col, n, engine):
                sq = scr.tile([128, n, C], F32, tag="sq")
                engine.tensor_mul(sq[:], x3[:], x3[:])
                nc.vector.reduce_sum(nrm[:, col : col + n], sq[:], axis=AX.X)

            def rsqrt_batch(col, n):
                nc.scalar.activation(nrm[:, col : col + n], nrm[:, col : col + n], AF.Ln)
                nc.scalar.activation(
                    inv[:, col : col + n], nrm[:, col : col + n], AF.Exp, scale=-0.5
                )

            # head-critical norms on DVE: q + chunks 0,1 unblock group 0
            norms(qt3, QC, MT, nc.vector)
            norms(kt3[:, 0:TPC, :], KC, TPC, nc.vector)
            norms(kt3[:, TPC : 2 * TPC, :], KC + TPC, TPC, nc.vector)
            rsqrt_batch(QC, 24)  # q + chunks 0,1

            # ---- normalize q (f32 kept for pos/corr), round to f32r, qT
            qn3 = big.tile([128, MT, C], F32)
            for t in range(MT):
                nc.vector.tensor_scalar_mul(
                    qn3[:, t, :], qt3[:, t, :], inv[:, t : t + 1]
                )
            qn3r = big.tile([128, MT, C], F32R)
            nc.vector.tensor_copy(qn3r[:], qn3[:])
            qT = big.tile([128, M_LOCAL], F32R)
            tq = psp.tile([128, SPAN], F32R, tag="ps")
            for t in range(MT):
                nc.tensor.transpose(
                    tq[:, t * 128 : (t + 1) * 128], qn3r[:, t, :], identr[:]
                )
            nc.vector.tensor_copy(qT[:], tq[:, 0:M_LOCAL])

            # squares for keys chunks 2-7 and p on otherwise-idle GpSimd;
            # the matching DVE reduces are emitted later (between groups)
            # to keep them out of the DVE queue ahead of group-0 scales
            gp_sq = []
            for g in range(2, NCHUNK):
                sqg = scr1.tile([128, TPC, C], F32, tag=f"gpsq{g}")
                nc.gpsimd.tensor_mul(sqg[:], kt3[:, g * TPC : (g + 1) * TPC, :],
                                     kt3[:, g * TPC : (g + 1) * TPC, :])
                gp_sq.append(sqg)
            sqp = scr1.tile([128, MT, C], F32, tag="sqp")
            nc.gpsimd.tensor_mul(sqp[:], pt3[:], pt3[:])

            # ---- main: per column group: scale+transpose 16 key tiles,
            # then 8 m-tiles of (4 matmuls + fused exp/rowsum)
            keysT = big.tile([128, TWOB], F32R)
            kn3 = big.tile([128, KT, C], F32R)
            acc = small.tile([128, MT * NG], F32)
            pn3 = big.tile([128, MT, C], F32)
            pos = small.tile([128, MT], F32)
            corr = small.tile([128, MT], F32)

            def transpose_group(G):
                for t in range(G * TPG, (G + 1) * TPG):
                    nc.vector.tensor_scalar_mul(
                        kn3[:, t, :], kt3[:, t, :], inv[:, KC + t : KC + t + 1]
                    )
                tp = psp.tile([128, SPAN], F32R, tag="ps")
                for i, t in enumerate(range(G * TPG, (G + 1) * TPG)):
                    nc.tensor.transpose(
                        tp[:, i * 128 : (i + 1) * 128], kn3[:, t, :], identr[:]
                    )
                nc.vector.tensor_copy(
                    keysT[:, G * SPAN : (G + 1) * SPAN], tp[:]
                )

            transpose_group(0)
            for G in range(NG):
                for m in range(MT):
                    psm = psp.tile([128, SPAN], F32, tag="ps")
                    for j in range(SPAN // 512):
                        col = G * SPAN + j * 512
                        nc.tensor.matmul(
                            psm[:, j * 512 : (j + 1) * 512],
                            lhsT=qT[:, m * 128 : (m + 1) * 128],
                            rhs=keysT[:, col : col + 512],
                            start=True,
                            stop=True,
                        )
                    nc.scalar.activation(
                        psm[:],
                        psm[:],
                        AF.Exp,
                        scale=ISCALE,
                        accum_out=acc[:, m * NG + G : m * NG + G + 1],
                    )
                    # between-group prep, interleaved mid-group so the
                    # DVE/ACT work overlaps this group's ScalarE spans
                    if m == 1 and G + 1 < NG:
                        g0, g1 = 2 * (G + 1), 2 * (G + 1) + 1
                        nc.vector.reduce_sum(
                            nrm[:, KC + g0 * TPC : KC + (g0 + 1) * TPC],
                            gp_sq[g0 - 2][:], axis=AX.X,
                        )
                        nc.vector.reduce_sum(
                            nrm[:, KC + g1 * TPC : KC + (g1 + 1) * TPC],
                            gp_sq[g1 - 2][:], axis=AX.X,
                        )
                        rsqrt_batch(KC + g0 * TPC, 2 * TPC)
                    if m == 3 and G + 1 < NG:
                        transpose_group(G + 1)
                    if G == 1 and m == 5:
                        # p path: needed only for the final pos/corr terms
                        nc.vector.reduce_sum(nrm[:, PC : PC + MT], sqp[:], axis=AX.X)
                        rsqrt_batch(PC, MT)
                        for t in range(MT):
                            nc.vector.tensor_scalar_mul(
                                pn3[:, t, :], pt3[:, t, :],
                                inv[:, PC + t : PC + t + 1],
                            )
                    if G == 2 and m == 5:
                        prod = scr.tile([128, MT, C], F32, tag="sq")
                        nc.vector.tensor_mul(prod[:], qn3[:], pn3[:])
                        nc.vector.reduce_sum(pos[:], prod[:], axis=AX.X)
                        prod2 = scr.tile([128, MT, C], F32, tag="sq")
                        nc.vector.tensor_mul(prod2[:], qn3[:], qn3[:])
                        nc.vector.reduce_sum(corr[:], prod2[:], axis=AX.X)

            # ---- finalize: loss_r = ln(S - e^{2 corr} + e^{2 pos}) - 2 pos
            S = small.tile([128, MT], F32)
            nc.vector.reduce_sum(
                S[:], acc[:].rearrange("p (m g) -> p m g", g=NG), axis=AX.X
            )
            ecorr = small.tile([128, MT], F32)
            epos = small.tile([128, MT], F32)
            nc.scalar.activation(ecorr[:], corr[:], AF.Exp, scale=ISCALE)
            nc.scalar.activation(epos[:], pos[:], AF.Exp, scale=ISCALE)
            tot = small.tile([128, MT], F32)
            nc.vector.tensor_sub(tot[:], S[:], ecorr[:])
            nc.vector.tensor_add(tot[:], tot[:], epos[:])
            nc.scalar.activation(tot[:], tot[:], AF.Ln)
            rowloss = small.tile([128, MT], F32)
            nc.vector.scalar_tensor_tensor(
                out=rowloss[:],
                in0=pos[:],
                scalar=-ISCALE,
                in1=tot[:],
                op0=ALU.mult,
                op1=ALU.add,
            )
            rsum = small.tile([128, 1], F32)
            nc.vector.reduce_sum(rsum[:], rowloss[:], axis=AX.X)
            nc.sync.dma_start(out=out[:], in_=rsum[:])

    nc.finalize()
    return nc


_NC_CACHE: bass.Bass | None = None
LAST_RESULTS = None  # BassKernelResults of the last run (for profiling)


def _get_nc() -> bass.Bass:
    global _NC_CACHE
    if _NC_CACHE is None:
        _NC_CACHE = build_bass()
    return _NC_CACHE


def kernel(z_i: np.ndarray, z_j: np.ndarray) -> np.ndarray:
    global LAST_RESULTS
    z_i = np.ascontiguousarray(np.asarray(z_i, dtype=np.float32))
    z_j = np.ascontiguousarray(np.asarray(z_j, dtype=np.float32))
    assert z_i.shape == (B, C) and z_j.shape == (B, C)

    reps = np.concatenate([z_i, z_j], axis=0)  # [2B, C]
    in_maps = []
    for k in range(N_CORES):
        kq = reps[k * M_LOCAL : (k + 1) * M_LOCAL]
        kp_blk = (k + N_CORES // 2) % N_CORES
        kp = reps[kp_blk * M_LOCAL : (kp_blk + 1) * M_LOCAL]
        in_maps.append(
            {
                "keys": reps,
                "q": np.ascontiguousarray(kq),
                "p": np.ascontiguousarray(kp),
            }
        )

    nc = _get_nc()
    trace = bool(int(os.environ.get("KERNEL_TRACE", "0")))
    res = run_bass_kernel_spmd(
        nc, in_maps, core_ids=list(range(N_CORES)), trace=trace
    )
    LAST_RESULTS = res
    total = sum(float(r["out"].sum()) for r in res.results)
    return np.float32(total / TWOB)


# revision 12
# speedup vs baseline: 1.6639x; 1.0191x over previous
"""NT-Xent loss kernel for Trainium2, 8-core SPMD.

Math (matches the reference exactly):
  reps = concat(z_i, z_j)                       [2B, C], B=4096, C=128
  rhat = reps / ||reps||                        (row L2 normalize)
  sim  = rhat @ rhat.T                          [2B, 2B]  (never materialized)
  pos_r = sim[r, (r+B) mod 2B]
  row logits = [pos_r, sim_r(with diag=-inf)] / T,  T=0.5
  loss = mean_r( logsumexp(row) - pos_r/T )
       = mean_r( ln(S_r - exp(2*d_r) + exp(2*pos_r)) - 2*pos_r )
  where S_r = sum_c exp(2 * rhat_r . rhat_c)  (includes diag + pos once)
        d_r = rhat_r . rhat_r  (~1; cancels the masked diagonal term)

Sharding: each of 8 cores owns 1024 query rows (contiguous block k) and
computes per-row (lse_r - 2*pos_r), reduced on-device to a [128,1]
per-partition partial; host sums 8x128 values / 2B. Core k's positive
partner block is block (k+4) % 8.

Perf design (v5; v1 fp32 baseline 202us, v4 124us):
  - matmul operands in float32r (~13-bit mantissa): 1 cyc/col vs 4 for
    fp32 -> 4x faster main loop, 2x faster PE transposes. pos/corr stay
    full fp32 on DVE.
  - query rows stay RAW: the 1/|q_r| factor rides the ScalarE exp's
    per-partition scale AP (out = exp(scale_r * raw_sim)), so qT needs
    no norms -> the q path runs as soon as its DMA lands.
  - exp row-sums via ScalarE accum_out (free reduce along keys).
  - one ACT table set (natural_log_exp_and_others) for Exp+Ln+rsqrt
    (rsqrt = exp(-0.5*ln)): a single ACT_TABLE_LOAD instead of 20.
  - input DMAs alternate the two HWDGE rings (sync + scalar).
  - all squares/reduces on DVE (GpSimd elementwise stalls DVE via SBUF
    port sharing -- measured 1.9us/op on both when concurrent); late
    chunks' norms are emitted mid-main-loop where DVE idles.
  - 16 transposes share one 4-bank PSUM tile -> one wide DVE copy each;
    group G+1 setup is emitted between group G's m-tiles.
"""

import os

import numpy as np

import concourse.bacc as bacc
import concourse.bass as bass
import concourse.mybir as mybir
from concourse.bass_utils import run_bass_kernel_spmd
from concourse.masks import make_identity
from concourse.tile import TileContext

F32 = mybir.dt.float32
F32R = mybir.dt.float32r
AF = mybir.ActivationFunctionType
ALU = mybir.AluOpType
AX = mybir.AxisListType

B = 4096
C = 128
TWOB = 2 * B            # 8192 total rows
N_CORES = 8
M_LOCAL = TWOB // N_CORES   # 1024 query rows per core
MT = M_LOCAL // 128         # 8 m-tiles of 128 queries
KT = TWOB // 128            # 64 key tiles of 128 rows
SPAN = 2048                 # ScalarE exp span = 4 PSUM banks
NG = TWOB // SPAN           # 4 column groups (16 key tiles each)
TPG = SPAN // 128           # 16 key tiles per column group
NCHUNK = 8                  # keys DMA chunks
TPC = KT // NCHUNK          # 8 key tiles per chunk
ISCALE = 2.0                # 1 / temperature

# nrm/inv column layout: 0:8 q | 8:72 keys (chunk g at 8+g*8) | 72:80 p
QC = 0
KC = MT
PC = MT + KT


def _patch_act_tables():
    """Leave Exp/Ln only in natural_log_exp_and_others so bacc's greedy
    set chooser emits ONE table load for the whole kernel (measured: the
    default choice alternated exp<->ln sets, 21 loads, ~27us)."""
    if getattr(bacc, "_ntx_act_patched", False):
        return
    orig = bacc.get_activation_tables

    def patched(arch):
        out = {}
        for name, fns in orig(arch).items():
            if name != "natural_log_exp_and_others":
                fns = fns - {AF.Exp, AF.Ln}
            out[name] = fns
        return out

    bacc.get_activation_tables = patched
    bacc._ntx_act_patched = True


def build_bass() -> bass.Bass:
    _patch_act_tables()
    nc = bacc.Bacc()
    keys = nc.dram_tensor("keys", [TWOB, C], F32, kind="ExternalInput")
    q = nc.dram_tensor("q", [M_LOCAL, C], F32, kind="ExternalInput")
    p = nc.dram_tensor("p", [M_LOCAL, C], F32, kind="ExternalInput")
    out = nc.dram_tensor("out", [128, 1], F32, kind="ExternalOutput")

    with TileContext(nc) as tc:
        with (
            tc.tile_pool(name="big", bufs=1) as big,
            tc.tile_pool(name="small", bufs=1) as small,
            tc.tile_pool(name="scr", bufs=2) as scr,
            tc.tile_pool(name="ps", bufs=2, space="PSUM") as psp,
        ):
            ident = small.tile([128, 128], F32)
            make_identity(nc, ident[:])
            identr = small.tile([128, 128], F32R)
            nc.vector.tensor_copy(identr[:], ident[:])  # round -> f32r

            # ---- input DMAs, alternating the two HWDGE rings
            qt3 = big.tile([128, MT, C], F32)
            pt3 = big.tile([128, MT, C], F32)
            kt3 = big.tile([128, KT, C], F32)
            nc.sync.dma_start(
                out=qt3[:], in_=q[:].rearrange("(t p) c -> p t c", p=128)
            )
            for g in range(NCHUNK):
                eng = nc.scalar if g % 2 else nc.sync
                eng.dma_start(
                    out=kt3[:, g * TPC : (g + 1) * TPC, :],
                    in_=keys[g * (TPC * 128) : (g + 1) * (TPC * 128), :].rearrange(
                        "(t p) c -> p t c", p=128
                    ),
                )
            nc.scalar.dma_start(
                out=pt3[:], in_=p[:].rearrange("(t p) c -> p t c", p=128)
            )

            # ---- q path: raw rows, rounded to f32r, transposed -> qT.
            # No norm dependency: 1/|q| is applied inside the main exp.
            qr3 = big.tile([128, MT, C], F32R)
            nc.vector.tensor_copy(qr3[:], qt3[:])
            qT = big.tile([128, M_LOCAL], F32R)
            tq = psp.tile([128, SPAN], F32R, tag="ps")
            for t in range(MT):
                nc.tensor.transpose(
                    tq[:, t * 128 : (t + 1) * 128], qr3[:, t, :], identr[:]
                )
            nc.vector.tensor_copy(qT[:], tq[:, 0:M_LOCAL])

            nrm = small.tile([128, 16 + KT], F32)
            inv = small.tile([128, 16 + KT], F32)

            def norms(x3, col, n):
                sq = scr.tile([128, n, C], F32, tag="sq")
                nc.vector.tensor_mul(sq[:], x3[:], x3[:])
                nc.vector.reduce_sum(nrm[:, col : col + n], sq[:], axis=AX.X)

            def rsqrt_batch(col, n):
                nc.scalar.activation(nrm[:, col : col + n], nrm[:, col : col + n], AF.Ln)
                nc.scalar.activation(
                    inv[:, col : col + n], nrm[:, col : col + n], AF.Exp, scale=-0.5
                )

            # head-critical: q (for the exp scale) + chunks 0,1 (group 0)
            norms(qt3, QC, MT)
            norms(kt3[:, 0:TPC, :], KC, TPC)
            norms(kt3[:, TPC : 2 * TPC, :], KC + TPC, TPC)
            rsqrt_batch(QC, 24)
            inv2q = small.tile([128, MT], F32)
            nc.vector.tensor_scalar_mul(inv2q[:], inv[:, 0:MT], ISCALE)

            # ---- main: per column group: scale+transpose 16 key tiles,
            # then 8 m-tiles of (4 matmuls + fused exp/rowsum); later
            # groups' norms and the p path are interleaved where DVE idles
            keysT = big.tile([128, TWOB], F32R)
            kn3 = big.tile([128, KT, C], F32R)
            acc = small.tile([128, MT * NG], F32)
            pos = small.tile([128, MT], F32)
            corr = small.tile([128, MT], F32)

            def transpose_group(G):
                for t in range(G * TPG, (G + 1) * TPG):
                    nc.vector.tensor_scalar_mul(
                        kn3[:, t, :], kt3[:, t, :], inv[:, KC + t : KC + t + 1]
                    )
                tp = psp.tile([128, SPAN], F32R, tag="ps")
                for i, t in enumerate(range(G * TPG, (G + 1) * TPG)):
                    nc.tensor.transpose(
                        tp[:, i * 128 : (i + 1) * 128], kn3[:, t, :], identr[:]
                    )
                nc.vector.tensor_copy(
                    keysT[:, G * SPAN : (G + 1) * SPAN], tp[:]
                )

            transpose_group(0)
            for G in range(NG):
                for m in range(MT):
                    psm = psp.tile([128, SPAN], F32, tag="ps")
                    for j in range(SPAN // 512):
                        col = G * SPAN + j * 512
                        nc.tensor.matmul(
                            psm[:, j * 512 : (j + 1) * 512],
                            lhsT=qT[:, m * 128 : (m + 1) * 128],
                            rhs=keysT[:, col : col + 512],
                            start=True,
                            stop=True,
                        )
                    nc.scalar.activation(
                        psm[:],
                        psm[:],
                        AF.Exp,
                        scale=inv2q[:, m : m + 1],
                        accum_out=acc[:, m * NG + G : m * NG + G + 1],
                    )
                    if m == 1 and G + 1 < NG:
                        g0 = 2 * (G + 1)
                        norms(kt3[:, g0 * TPC : (g0 + 2) * TPC, :], KC + g0 * TPC,
                              2 * TPC)
                        rsqrt_batch(KC + g0 * TPC, 2 * TPC)
                    if m == 3 and G + 1 < NG:
                        transpose_group(G + 1)
                    if G == 1 and m == 5:
                        # p path: only needed for the final pos term
                        norms(pt3, PC, MT)
                        rsqrt_batch(PC, MT)
                    if G == 2 and m == 5:
                        # pos/corr from RAW dots times the inverse norms
                        prod = scr.tile([128, MT, C], F32, tag="sq")
                        nc.vector.tensor_mul(prod[:], qt3[:], pt3[:])
                        nc.vector.reduce_sum(pos[:], prod[:], axis=AX.X)
                        prod2 = scr.tile([128, MT, C], F32, tag="sq")
                        nc.vector.tensor_mul(prod2[:], qt3[:], qt3[:])
                        nc.vector.reduce_sum(corr[:], prod2[:], axis=AX.X)
                        nc.vector.tensor_mul(pos[:], pos[:], inv[:, QC : QC + MT])
                        nc.vector.tensor_mul(pos[:], pos[:], inv[:, PC : PC + MT])
                        nc.vector.tensor_mul(corr[:], corr[:], inv[:, QC : QC + MT])
                        nc.vector.tensor_mul(corr[:], corr[:], inv[:, QC : QC + MT])

            # ---- finalize: loss_r = ln(S - e^{2 corr} + e^{2 pos}) - 2 pos
            S = small.tile([128, MT], F32)
            nc.vector.reduce_sum(
                S[:], acc[:].rearrange("p (m g) -> p m g", g=NG), axis=AX.X
            )
            ecorr = small.tile([128, MT], F32)
            epos = small.tile([128, MT], F32)
            nc.scalar.activation(ecorr[:], corr[:], AF.Exp, scale=ISCALE)
            nc.scalar.activation(epos[:], pos[:], AF.Exp, scale=ISCALE)
            tot = small.tile([128, MT], F32)
            nc.vector.tensor_sub(tot[:], S[:], ecorr[:])
            nc.vector.tensor_add(tot[:], tot[:], epos[:])
            nc.scalar.activation(tot[:], tot[:], AF.Ln)
            rowloss = small.tile([128, MT], F32)
            nc.vector.scalar_tensor_tensor(
                out=rowloss[:],
                in0=pos[:],
                scalar=-ISCALE,
                in1=tot[:],
                op0=ALU.mult,
                op1=ALU.add,
            )
            rsum = small.tile([128, 1], F32)
            nc.vector.reduce_sum(rsum[:], rowloss[:], axis=AX.X)
            nc.sync.dma_start(out=out[:], in_=rsum[:])

    nc.finalize()
    return nc


_NC_CACHE: bass.Bass | None = None
LAST_RESULTS = None  # BassKernelResults of the last run (for profiling)


def _get_nc() -> bass.Bass:
    global _NC_CACHE
    if _NC_CACHE is None:
        _NC_CACHE = build_bass()
    return _NC_CACHE


def kernel(z_i: np.ndarray, z_j: np.ndarray) -> np.ndarray:
    global LAST_RESULTS
    z_i = np.ascontiguousarray(np.asarray(z_i, dtype=np.float32))
    z_j = np.ascontiguousarray(np.asarray(z_j, dtype=np.float32))
    assert z_i.shape == (B, C) and z_j.shape == (B, C)

    reps = np.concatenate([z_i, z_j], axis=0)  # [2B, C]
    in_maps = []
    for k in range(N_CORES):
        kq = reps[k * M_LOCAL : (k + 1) * M_LOCAL]
        kp_blk = (k + N_CORES // 2) % N_CORES
        kp = reps[kp_blk * M_LOCAL : (kp_blk + 1) * M_LOCAL]
        in_maps.append(
            {
                "keys": reps,
                "q": np.ascontiguousarray(kq),
                "p": np.ascontiguousarray(kp),
            }
        )

    nc = _get_nc()
    trace = bool(int(os.environ.get("KERNEL_TRACE", "0")))
    res = run_bass_kernel_spmd(
        nc, in_maps, core_ids=list(range(N_CORES)), trace=trace
    )
    LAST_RESULTS = res
    total = sum(float(r["out"].sum()) for r in res.results)
    return np.float32(total / TWOB)
